# revision 4
# baseline (speedup 1.0000x reference)
"""Dynamic per-sample CNN (nn_ConvFunc) Trainium2 Bass kernel.

Reference computation (per sample b):
  cnn_inp = proj_w @ cat(lhs, rhs) + proj_b          # 1x1 conv, [128, 32, 32]
  out     = conv3x3(cnn_inp, W_b) + bias_b           # W_b, bias_b unpacked from question_rep[b]

Sharding: pure data parallel, 8 samples per NeuronCore (batch 64 / 8 cores).

Per-core device kernel, per sample:
  - proj: for each 512-col half of the 32x32=1024 pixel space, 2 accumulating
    fp32r matmuls (lhs-channels, rhs-channels) into one PSUM bank; DVE evicts
    PSUM (+proj_b per-partition bias) into the interior of a zeroed [128,34,34]
    padded SBUF tile.
  - conv: for each half (16 output rows), 9 accumulating fp32r matmuls, one per
    (kh, kw) tap, rhs = shifted 16x32 window of the padded tile (strided AP);
    DVE evicts PSUM (+cnn bias) to SBUF, DMA to DRAM.

Weights are host-pretransposed so each tap's [128(in), 128(out)] block is a
ready-to-use lhsT tile and the per-sample weight DMA is fully contiguous.
"""

import numpy as np

import concourse.bass as bass
import concourse.mybir as mybir
from concourse import bacc
from concourse.tile import TileContext
from concourse.bass_utils import run_bass_kernel_spmd

# Problem shapes (hardcoded per contract)
B = 64
DIM = 128
H = W = 32
K = 3
KK = K * K
HW = H * W           # 1024
WDIM = DIM * DIM * KK  # 147456
NCORES = 8
SPC = B // NCORES    # samples per core
HP, WP = H + 2, W + 2  # padded 34x34
HALF = HW // 2       # 512 columns per PSUM bank
HROWS = H // 2       # 16 output rows per half

FP = mybir.dt.float32
FR = mybir.dt.float32r

_BUILT = {}


def _fr(ap):
    return ap.bitcast(FR)


def build_nc(mm_dt=FR):
    nc = bacc.Bacc("TRN2", target_bir_lowering=False, debug=False,
                   num_devices=NCORES)

    qw = nc.declare_dram_parameter("qw", [SPC, DIM, KK * DIM], FP, isOutput=False)
    qb = nc.declare_dram_parameter("qb", [DIM, SPC], FP, isOutput=False)
    xl = nc.declare_dram_parameter("xl", [SPC, DIM, HW], FP, isOutput=False)
    xr = nc.declare_dram_parameter("xr", [SPC, DIM, HW], FP, isOutput=False)
    pw = nc.declare_dram_parameter("pw", [2, DIM, DIM], FP, isOutput=False)
    pb = nc.declare_dram_parameter("pb", [DIM, 1], FP, isOutput=False)
    out = nc.declare_dram_parameter("out", [SPC, DIM, HW], FP, isOutput=True)

    with TileContext(nc) as tc:
        with (
            tc.tile_pool(name="const", bufs=1) as cpool,
            tc.tile_pool(name="wpool", bufs=3) as wpool,
            tc.tile_pool(name="xpool", bufs=3) as xpool,
            tc.tile_pool(name="xppool", bufs=3) as xppool,
            tc.tile_pool(name="opool", bufs=3) as opool,
            tc.tile_pool(name="pp_pool", bufs=2, space="PSUM") as pp_pool,
            tc.tile_pool(name="pc_pool", bufs=4, space="PSUM") as pc_pool,
        ):
            pw_sb = cpool.tile([DIM, 2, DIM], mm_dt)
            nc.gpsimd.dma_start(out=pw_sb[:, 0:1, :], in_=pw[0:1])
            nc.gpsimd.dma_start(out=pw_sb[:, 1:2, :], in_=pw[1:2])
            qb_sb = cpool.tile([DIM, SPC], FP)
            nc.sync.dma_start(out=qb_sb[:], in_=qb[:])
            pb_sb = cpool.tile([DIM, 1], FP)
            nc.sync.dma_start(out=pb_sb[:], in_=pb[:])

            def proj(s):
                xl_sb = xpool.tile([DIM, HW], mm_dt, tag="xl")
                nc.gpsimd.dma_start(out=xl_sb[:], in_=xl[s])
                xr_sb = xpool.tile([DIM, HW], mm_dt, tag="xr")
                nc.gpsimd.dma_start(out=xr_sb[:], in_=xr[s])
                xp = xppool.tile([DIM, HP, WP], mm_dt, tag="xp")
                nc.vector.memset(xp[:, 0:1, :].bitcast(FP), 0.0)
                nc.vector.memset(xp[:, HP - 1:HP, :].bitcast(FP), 0.0)
                nc.vector.memset(xp[:, 1:HP - 1, 0:1].bitcast(FP), 0.0)
                nc.vector.memset(xp[:, 1:HP - 1, WP - 1:WP].bitcast(FP), 0.0)
                for h in range(2):
                    ppt = pp_pool.tile([DIM, HALF], FP, tag="pp")
                    nc.tensor.matmul(ppt[:], lhsT=pw_sb[:, 0, :],
                                     rhs=xl_sb[:, h * HALF:(h + 1) * HALF],
                                     start=True, stop=False)
                    nc.tensor.matmul(ppt[:], lhsT=pw_sb[:, 1, :],
                                     rhs=xr_sb[:, h * HALF:(h + 1) * HALF],
                                     start=False, stop=True)
                    nc.vector.tensor_scalar_add(
                        xp[:, 1 + HROWS * h:1 + HROWS * (h + 1), 1:1 + W],
                        ppt[:].rearrange("p (a b) -> p a b", b=W),
                        pb_sb[:, 0:1],
                    )
                return xp

            def load_w(s):
                w_sb = wpool.tile([DIM, KK, DIM], mm_dt, tag="w")
                nc.gpsimd.dma_start(out=w_sb[:], in_=qw[s])
                return w_sb

            def conv(s, xp, w_sb):
                o_sb = opool.tile([DIM, HW], FP, tag="o")
                for h in range(2):
                    pct = pc_pool.tile([DIM, HALF], FP, tag="pc")
                    t = 0
                    for kh in range(K):
                        for kw in range(K):
                            nc.tensor.matmul(
                                pct[:],
                                lhsT=w_sb[:, kh * K + kw, :],
                                rhs=xp[:, HROWS * h + kh:HROWS * (h + 1) + kh,
                                           kw:kw + W],
                                start=(t == 0), stop=(t == KK - 1))
                            t += 1
                    nc.vector.tensor_scalar_add(
                        o_sb[:, h * HALF:(h + 1) * HALF], pct[:], qb_sb[:, s:s + 1])
                nc.sync.dma_start(out=out[s], in_=o_sb[:])

            # software pipeline: proj(s) ahead of conv(s-1) keeps PE dense
            prev = None
            for s in range(SPC):
                xp = proj(s)
                w_sb = load_w(s)
                if prev is not None:
                    conv(*prev)
                prev = (s, xp, w_sb)
            conv(*prev)

    nc.compile()
    return nc


def _prep(question_rep, lhs_rep, rhs_rep, proj_w, proj_b):
    """Host-side shard + layout prep (cheap reshapes/transposes only)."""
    qr = np.ascontiguousarray(question_rep, dtype=np.float32)
    # conv weights: [B, o, i, kh, kw] -> [B, i, (kh kw), o] so each tap is a
    # ready lhsT [i, o] block and the per-sample DMA is contiguous
    qw = qr[:, :WDIM].reshape(B, DIM, DIM, K, K).transpose(0, 2, 3, 4, 1)
    qw = np.ascontiguousarray(qw).reshape(B, DIM, KK * DIM)
    qb = np.ascontiguousarray(qr[:, WDIM:])             # [B, 128]
    xl = np.ascontiguousarray(lhs_rep, dtype=np.float32).reshape(B, DIM, HW)
    xr = np.ascontiguousarray(rhs_rep, dtype=np.float32).reshape(B, DIM, HW)
    pwt = np.ascontiguousarray(np.asarray(proj_w, dtype=np.float32).T)  # [256, 128]
    pw = pwt.reshape(2, DIM, DIM)
    pb = np.ascontiguousarray(np.asarray(proj_b, dtype=np.float32).reshape(DIM, 1))

    in_maps = []
    for c in range(NCORES):
        sl = slice(c * SPC, (c + 1) * SPC)
        in_maps.append({
            "qw": np.ascontiguousarray(qw[sl]),
            "qb": np.ascontiguousarray(qb[sl].T),       # [128, SPC]
            "xl": xl[sl],
            "xr": xr[sl],
            "pw": pw,
            "pb": pb,
        })
    return in_maps


def kernel(question_rep, lhs_rep, rhs_rep, proj_w, proj_b, _run_kwargs=None):
    if "nc" not in _BUILT:
        _BUILT["nc"] = build_nc()
    nc = _BUILT["nc"]
    in_maps = _prep(question_rep, lhs_rep, rhs_rep, proj_w, proj_b)
    res = run_bass_kernel_spmd(nc, in_maps, core_ids=list(range(NCORES)),
                               **(_run_kwargs or {}))
    out = np.concatenate([res.results[c]["out"] for c in range(NCORES)], axis=0)
    if _run_kwargs is not None:
        _BUILT["last_result"] = res
    return out.reshape(B, DIM, H, W)


if __name__ == "__main__":
    rng = np.random.default_rng(0)
    inputs = {
        "question_rep": rng.standard_normal((B, WDIM + DIM), dtype=np.float32) * 0.05,
        "lhs_rep": rng.standard_normal((B, DIM, H, W), dtype=np.float32),
        "rhs_rep": rng.standard_normal((B, DIM, H, W), dtype=np.float32),
        "proj_w": rng.standard_normal((DIM, 2 * DIM), dtype=np.float32),
        "proj_b": rng.standard_normal((DIM,), dtype=np.float32) * 0.01,
    }
    out = kernel(**inputs)
    print("ran, out shape:", out.shape)
